# revision 7
# baseline (speedup 1.0000x reference)
"""Trainium2 Bass kernel for 16-head causal MHA (B=4, S=2048, D=1024).

Sharding: 8 cores = 4 batches x 2 head-groups (8 heads each).
Per core: fused QKV projections (fp32r matmuls), causal attention computed in
BOTH orientations (scoresT for A*V + denominators via a ones-augmented V;
natural orientation for the softmax-weights output, with the normalization
folded into the exp bias), then the output projection. Host sums the two
head-group partials per batch and adds the bias.
"""

import numpy as np

HEAD = 16
HEAD_DIM = 64
DIM = 1024
B = 4
S = 2048
NEG = -1000000000.0

HG = 8          # heads per core
NJ = 16         # 128-row j chunks
NI = 4          # 512-col i tiles
NST = 16        # 128-row s tiles


def _round_f32r(x):
    """Round fp32 to the 10-bit-mantissa fp32r format (round-to-nearest)."""
    u = np.ascontiguousarray(x, dtype=np.float32).view(np.uint32)
    u = (u + np.uint32(0x1000)) & np.uint32(0xFFFFE000)
    return u.view(np.float32)


def _split_multiwaits(nc, max_waits=1):
    """This walrus build rejects instructions with >1 sem wait: move excess
    waits onto same-engine NoOps inserted just before the instruction."""
    import concourse.mybir as mybir

    for fn in nc.m.functions:
        for bb in fn.blocks:
            new_insts = []
            for inst in bb.instructions:
                si = inst.sync_info
                if si is not None and si.on_wait and len(si.on_wait) > max_waits:
                    waits = list(si.on_wait)
                    excess, keep = waits[:-max_waits], waits[-max_waits:]
                    for w in excess:
                        nop = mybir.InstNoOp(
                            name=nc.get_next_instruction_name(),
                            engine=inst.engine,
                            ins=[],
                            outs=[],
                            sync_info=mybir.SyncInfo(on_wait=[w], on_update=[]),
                        )
                        nc.register_instruction(nop, overwrite=True)
                        new_insts.append(nop)
                    si.on_wait = keep
                new_insts.append(inst)
            bb.instructions = new_insts


def _build_program():
    import os
    import concourse.bass as bass
    import concourse.mybir as mybir
    from concourse.tile import TileContext

    f32 = mybir.dt.float32
    f32r = mybir.dt.float32r
    bf16 = mybir.dt.bfloat16
    EXPF = mybir.ActivationFunctionType.Exp
    LNF = mybir.ActivationFunctionType.Ln

    nc = bass.Bass("TRN2", target_bir_lowering=False, debug=False, num_devices=8)

    xtq = nc.dram_tensor("xtq", [DIM, S], f32r, kind="ExternalInput")
    xtk = nc.dram_tensor("xtk", [DIM, S], f32r, kind="ExternalInput")
    xtv = nc.dram_tensor("xtv", [DIM, S], f32r, kind="ExternalInput")
    wqt = nc.dram_tensor("wqt", [DIM, 512], f32r, kind="ExternalInput")
    wkt = nc.dram_tensor("wkt", [DIM, 512], f32r, kind="ExternalInput")
    wvt = nc.dram_tensor("wvt", [DIM, 512], f32r, kind="ExternalInput")
    wot = nc.dram_tensor("wot", [512, DIM], f32r, kind="ExternalInput")
    mskt = nc.dram_tensor("mskt", [4, 128, 512], f32, kind="ExternalInput")
    mskn = nc.dram_tensor("mskn", [4, 128, 512], f32, kind="ExternalInput")
    w_out = nc.dram_tensor("w_out", [HG, S, S], f32, kind="ExternalOutput")
    o_out = nc.dram_tensor("o_out", [S, DIM], f32, kind="ExternalOutput")

    with TileContext(nc) as tc:
        with tc.tile_pool(name="main", bufs=1) as mainp:
            qT = mainp.tile([128, 4, S], f32r)       # [hd%128, hd//128, s]
            kT = mainp.tile([128, 4, S], f32r)
            vaug = mainp.tile([128, NJ, HG * 66], bf16)  # [j%128, j//128, h*66+(d|1)]
            outT = mainp.tile([128, 4, S], f32r)

            phases = os.environ.get("KERNEL_PHASES", "123")
            # ---------------- Phase 1: projections ----------------
            with tc.tile_pool(name="p1", bufs=1) as p1, \
                 tc.tile_pool(name="ps1", bufs=1, space="PSUM") as ps1:
                # q and k: out tiles (hd 128, s 512), contraction over dim
                for dst, xsrc, wsrc in ((qT, xtq, wqt), (kT, xtk, wkt)):
                    wt = p1.tile([128, 8, 512], f32r, tag="wt")
                    nc.sync.dma_start(out=wt[:], in_=wsrc.rearrange("(ko ki) h -> ki ko h", ki=128))
                    for n in range(4):
                        psq = [ps1.tile([128, 512], f32, tag="psq", bufs=6, name=f"psq{m}") for m in range(4)]
                        for k in range(8):
                            xc = p1.tile([128, 512], f32r, tag="xtnk", bufs=4)
                            nc.sync.dma_start(out=xc[:], in_=xsrc[k * 128:(k + 1) * 128, n * 512:(n + 1) * 512])
                            for m in range(4):
                                nc.tensor.matmul(psq[m][:], wt[:, k, m * 128:(m + 1) * 128], xc[:],
                                                 start=(k == 0), stop=(k == 7))
                        for m in range(4):
                            nc.vector.tensor_copy(dst[:, m, n * 512:(n + 1) * 512], psq[m][:])

                # v: natural layout (s, hd) in two k-waves, assembled into vaug
                wtv = p1.tile([128, 8, 512], f32r, tag="wt")
                nc.sync.dma_start(out=wtv[:], in_=wvt.rearrange("(ko ki) h -> ki ko h", ki=128))
                ones2 = p1.tile([128, 2], f32, tag="ones2")
                nc.gpsimd.memset(ones2[:], 1.0)
                vA = p1.tile([128, NST, 512], bf16, tag="vA")
                vh = [None, None]
                for half in range(2):
                    vh[half] = p1.tile([128, 4, S], f32r, tag="vhalf", name=f"vh{half}")
                    for kl in range(4):
                        kk = half * 4 + kl
                        nc.sync.dma_start(out=vh[half][:, kl, :], in_=xtv[kk * 128:(kk + 1) * 128, :])
                for st in range(NST if "1" in phases else 0):
                    psv = ps1.tile([128, 512], f32, tag="psv", bufs=2)
                    for kl in range(4):
                        nc.tensor.matmul(psv[:], vh[0][:, kl, st * 128:(st + 1) * 128], wtv[:, kl, :],
                                         start=(kl == 0), stop=(kl == 3))
                    nc.vector.tensor_copy(vA[:, st, :], psv[:])
                for st in range(NST):
                    psv = ps1.tile([128, 512], f32, tag="psv", bufs=2)
                    for kl in range(4):
                        nc.tensor.matmul(psv[:], vh[1][:, kl, st * 128:(st + 1) * 128], wtv[:, 4 + kl, :],
                                         start=(kl == 0), stop=(kl == 3))
                    for h in range(HG):
                        nc.vector.tensor_tensor(vaug[:, st, h * 66:h * 66 + 64],
                                                psv[:, h * 64:(h + 1) * 64],
                                                vA[:, st, h * 64:(h + 1) * 64],
                                                mybir.AluOpType.add)
                    for h in range(HG):
                        nc.vector.tensor_copy(vaug[:, st, h * 66 + 64:h * 66 + 66], ones2[:])

            # ---------------- Phase 2: attention ----------------
            with tc.tile_pool(name="p2", bufs=1) as p2, \
                 tc.tile_pool(name="ps2", bufs=1, space="PSUM") as ps2:
                mT = p2.tile([128, 4, 512], f32)
                nc.sync.dma_start(out=mT[:], in_=mskt[:].rearrange("m p c -> p m c"))
                mN = p2.tile([128, 4, 512], f32)
                nc.sync.dma_start(out=mN[:], in_=mskn[:].rearrange("m p c -> p m c"))
                onesk = p2.tile([1, 2], f32)
                nc.gpsimd.memset(onesk[:], 1.0)
                ones64 = p2.tile([1, 64], f32)
                nc.gpsimd.memset(ones64[:], 1.0)

                for p in range(4 if "2" in phases else 0):   # head pair
                    for it in range(NI):    # i tile of 512
                        i0 = it * 512
                        nj = 4 * (it + 1)
                        psAV = [ps2.tile([128, 512], f32, tag="psav", bufs=2, name=f"psav{h2}") for h2 in range(2)]
                        for j in range(nj):
                            psT = ps2.tile([128, 1024], f32, tag="pst", bufs=2)
                            for h2 in range(2):
                                base = 64 * h2
                                nc.tensor.matmul(psT[:, h2 * 512:(h2 + 1) * 512],
                                                 kT[base:base + 64, p, j * 128:(j + 1) * 128],
                                                 qT[base:base + 64, p, i0:i0 + 512],
                                                 start=True, stop=True, tile_position=(base, 0))
                            m = j - 4 * it
                            if m >= 0:      # crossing tile: pre-exp NEG mask
                                w = 128 * (m + 1)
                                for h2 in range(2):
                                    nc.vector.tensor_tensor(psT[:, h2 * 512:h2 * 512 + w],
                                                            psT[:, h2 * 512:h2 * 512 + w],
                                                            mT[:, m, 0:w], mybir.AluOpType.add)
                            ebf = p2.tile([128, 1024], bf16, tag="ebf", bufs=4)
                            nc.scalar.activation(ebf[:], psT[:], EXPF, scale=0.125)
                            for h2 in range(2):
                                h = 2 * p + h2
                                nc.tensor.matmul(psAV[h2][0:66, :],
                                                 vaug[:, j, h * 66:(h + 1) * 66],
                                                 ebf[:, h2 * 512:(h2 + 1) * 512],
                                                 start=(j == 0), stop=(j == nj - 1))
                        for h2 in range(2):
                            base = 64 * h2
                            h = 2 * p + h2
                            dsb = p2.tile([1, 512], f32, tag="dsb", bufs=4)
                            nc.vector.tensor_copy(dsb[:], psAV[h2][64:65, :])
                            rsb = p2.tile([1, 512], f32, tag="rsb", bufs=4)
                            nc.vector.reciprocal(rsb[:], dsb[:])
                            # replicate 1/denom across 64 partitions via K=1 matmul
                            psR = ps2.tile([128, 512], f32, tag="psn", bufs=2, name="psR")
                            nc.tensor.matmul(psR[0:64, :], ones64[:], rsb[:],
                                             start=True, stop=True)
                            rb64 = p2.tile([64, 512], f32, tag="rb64", bufs=4)
                            nc.vector.tensor_copy(rb64[:], psR[0:64, :])
                            nc.vector.tensor_tensor(outT[base:base + 64, p, i0:i0 + 512],
                                                    psAV[h2][0:64, :],
                                                    rb64[:],
                                                    mybir.AluOpType.mult)
                            lsb = p2.tile([1, 512], f32, tag="lsb", bufs=4)
                            nc.scalar.activation(lsb[:], rsb[:], LNF)   # -ln(denom)
                            psB = ps2.tile([128, 512], f32, tag="psn", bufs=2)
                            for c in range(4):
                                nc.tensor.matmul(psB[:, 2 * c:2 * c + 2],
                                                 lsb[:, c * 128:(c + 1) * 128], onesk[:],
                                                 start=True, stop=True)
                            biasb = p2.tile([128, 4], f32, tag="biasb", bufs=4)
                            nc.vector.tensor_copy(biasb[:], psB[:, 0:8:2])
                            for i4 in range(4):
                                isg = 4 * it + i4
                                for jb in range(it + 1):
                                    psN = ps2.tile([128, 512], f32, tag="psn", bufs=2)
                                    nc.tensor.matmul(psN[:],
                                                     qT[base:base + 64, p, isg * 128:(isg + 1) * 128],
                                                     kT[base:base + 64, p, jb * 512:(jb + 1) * 512],
                                                     start=True, stop=True, tile_position=(base, 0))
                                    if jb == it:
                                        c0 = 128 * i4
                                        nc.vector.tensor_tensor(psN[:, c0:512], psN[:, c0:512],
                                                                mN[:, i4, c0:512], mybir.AluOpType.add)
                                    wsb = p2.tile([128, 512], f32, tag="wsb", bufs=6)
                                    nc.scalar.activation(wsb[:], psN[:], EXPF,
                                                         scale=0.125, bias=biasb[:, i4:i4 + 1])
                                    nc.sync.dma_start(
                                        out=w_out[h, isg * 128:(isg + 1) * 128, jb * 512:(jb + 1) * 512],
                                        in_=wsb[:])

            # ---------------- Phase 3: output projection ----------------
            with tc.tile_pool(name="p3", bufs=1) as p3, \
                 tc.tile_pool(name="ps3", bufs=1, space="PSUM") as ps3:
                wo = p3.tile([128, 4, DIM], f32r)
                nc.sync.dma_start(out=wo[:], in_=wot.rearrange("(ko ki) d -> ki ko d", ki=128))
                for mst in range(NST if "3" in phases else 0):
                    pso = [ps3.tile([128, 512], f32, tag="pso", bufs=4, name=f"pso{n}") for n in range(2)]
                    for k in range(4):
                        for n in range(2):
                            nc.tensor.matmul(pso[n][:], outT[:, k, mst * 128:(mst + 1) * 128],
                                             wo[:, k, n * 512:(n + 1) * 512],
                                             start=(k == 0), stop=(k == 3))
                    osb = p3.tile([128, 1024], f32, tag="osb", bufs=4)
                    for n in range(2):
                        nc.vector.tensor_copy(osb[:, n * 512:(n + 1) * 512], pso[n][:])
                    nc.sync.dma_start(out=o_out[mst * 128:(mst + 1) * 128, :], in_=osb[:])

    _split_multiwaits(nc)
    return nc


_CACHED = {}


def _masks():
    r = np.arange(128)[:, None]
    c = np.arange(512)[None, :]
    mt = np.zeros((4, 128, 512), np.float32)
    mn = np.zeros((4, 128, 512), np.float32)
    for m in range(4):
        # T-side tile (j=r, i=c), offset i0-j0 = -128m: invalid iff c < r + 128m
        mt[m][c < r + 128 * m] = NEG
        # natural tile (i=r, j=c), offset i0-j0 = 128m: invalid iff c > r + 128m
        mn[m][c > r + 128 * m] = NEG
    return mt, mn


def kernel(Q, K, V, Wq, Wk, Wv, Wo, bo):
    out, weights, _ = _run(Q, K, V, Wq, Wk, Wv, Wo, bo)
    return out, weights


def _run(Q, K, V, Wq, Wk, Wv, Wo, bo, **run_kwargs):
    from concourse.bass_utils import run_bass_kernel_spmd

    if "nc" not in _CACHED:
        _CACHED["nc"] = _build_program()
    nc = _CACHED["nc"]

    Q = np.asarray(Q, np.float32)
    K = np.asarray(K, np.float32)
    V = np.asarray(V, np.float32)
    Wq = np.asarray(Wq, np.float32)
    Wk = np.asarray(Wk, np.float32)
    Wv = np.asarray(Wv, np.float32)
    Wo = np.asarray(Wo, np.float32)
    bo = np.asarray(bo, np.float32)

    mt, mn = _masks()
    xt = {}
    for b in range(B):
        xt[b] = (
            _round_f32r(np.ascontiguousarray(Q[b].T)),
            _round_f32r(np.ascontiguousarray(K[b].T)),
            _round_f32r(np.ascontiguousarray(V[b].T)),
        )
    wslice = {}
    for g in range(2):
        hs = slice(g * 512, (g + 1) * 512)
        wslice[g] = (
            _round_f32r(np.ascontiguousarray(Wq[hs].T)),
            _round_f32r(np.ascontiguousarray(Wk[hs].T)),
            _round_f32r(np.ascontiguousarray(Wv[hs].T)),
            _round_f32r(np.ascontiguousarray(Wo[:, hs].T)),
        )

    in_maps = []
    for c in range(8):
        b, g = c // 2, c % 2
        wq, wk, wv, wo = wslice[g]
        in_maps.append({
            "xtq": xt[b][0], "xtk": xt[b][1], "xtv": xt[b][2],
            "wqt": wq, "wkt": wk, "wvt": wv, "wot": wo,
            "mskt": mt, "mskn": mn,
        })

    r = run_bass_kernel_spmd(nc, in_maps, list(range(8)), **run_kwargs)

    weights = np.empty((B, HEAD, S, S), np.float32)
    out = np.empty((B, S, DIM), np.float32)
    for b in range(B):
        for g in range(2):
            res = r.results[2 * b + g]
            weights[b, g * HG:(g + 1) * HG] = res["w_out"]
        out[b] = r.results[2 * b]["o_out"] + r.results[2 * b + 1]["o_out"] + bo
    return out, weights, r
